# revision 22
# baseline (speedup 1.0000x reference)
"""GQA causal attention (S=2048, H=32, KVH=8, D=128) on 8 TRN2 NeuronCores.

Sharding: tensor-parallel over heads. Core i computes query heads
[4i, 4i+4) against KV head i (GQA group size 32/8 = 4). No collectives:
the host slices the inputs per core and concatenates the outputs.

Per-core algorithm (seq=2048, d=128, 4 q-heads, 1 kv-head, causal):
  - K^T and head-0 Q^T take the low-latency prep path: fp32 DMA load,
    DVE cast to bf16, PE identity-matmul transpose (chunked, interleaved
    with warmup matmuls that hold the HAM clock at 2.4 GHz).
  - Q^T for heads 1-3 is produced entirely by DMA engines in the
    background: a SWDGE DMA casts fp32->bf16 (DRAM->DRAM), then an
    XBAR-transpose DMA lands [d=128, seq] bf16 in SBUF. These are issued
    at t=0 and consumed 25+ us later.
  - Per head, exact-causal score tiles S^T[kt] = K_tile^T @ Q^T (only
    q >= kt*128) are written PACKED into alternating PSUM buffers
    A[128,2048] / B[128,1024]; ONE wide ACTIVATE(Exp, scale) per buffer
    writes into the packed P^T buffer [128, 17408] bf16 (scores are O(1)
    so no max subtraction). 44 activations instead of 96.
  - The diagonal 128-col block of each key-tile region is masked by a
    0/1 upper-triangular multiply on the (otherwise idle) GpSimd engine.
  - PV: for each query tile qt, acc[qt] = sum_k2 (P^T slice).T @ [V | 1]
    accumulated in PSUM; column 128 is the softmax denominator.
    DVE reciprocal + tensor_scalar_mul normalizes; one DMA per 256 rows
    stores the result. PV lags the QK/exp pipeline by a few query tiles
    and flows across head boundaries so no engine sees a bubble.
"""

import numpy as np

SEQ = 2048
D = 128
QH = 4  # query heads per core
N_CORES = 8
SCALE = 0.08838834764831845  # 1/sqrt(128)
NT = SEQ // 128  # 16 tiles of 128 along seq

_NC = None

# packed score-column layout (identical per head)
ROFF = [0]
for _kt in range(1, NT + 1):
    ROFF.append(ROFF[-1] + (SEQ - 128 * (_kt - 1)))
PCOLS = ROFF[NT]  # 17408

# psum buffers: B(1024) first so the head's first ACTIVATE has a short
# dependency chain, then alternate with A(2048); the tail is one extra A.
_SIZES = [1024, 2048] * 5 + [2048]  # sums to PCOLS
BUFS = []
_c = 0
for _sz in _SIZES:
    BUFS.append((_c, _sz, 1 if _sz == 1024 else 0))  # (start, size, pool: 0=A,1=B)
    _c += _sz
assert _c == PCOLS


def _emit(ctx, tc, q, k, v, out):
    import concourse.mybir as mybir
    from concourse import masks

    nc = tc.nc
    f32 = mybir.dt.float32
    bf16 = mybir.dt.bfloat16
    Exp = mybir.ActivationFunctionType.Exp

    singles = ctx.enter_context(tc.tile_pool(name="singles", bufs=1))
    ppool = ctx.enter_context(tc.tile_pool(name="ppool", bufs=2))
    opool = ctx.enter_context(tc.tile_pool(name="opool", bufs=3))
    # PSUM budget (8 banks = 16KB/partition):
    #   A 2048 f32 = 4 banks, B 1024 f32 = 2 banks,
    #   PV acc [128,2,129] f32 = 1 bank, transpose staging = 1 bank
    psum_a = ctx.enter_context(tc.tile_pool(name="psum_a", bufs=1, space="PSUM"))
    psum_b = ctx.enter_context(tc.tile_pool(name="psum_b", bufs=1, space="PSUM"))
    psum_o = ctx.enter_context(tc.tile_pool(name="psum_o", bufs=1, space="PSUM"))
    psum_t = ctx.enter_context(tc.tile_pool(name="psum_t", bufs=1, space="PSUM"))

    sA = psum_a.tile([128, 2048], f32, tag="A")
    sB = psum_b.tile([128, 1024], f32, tag="B")
    ops_tri = psum_o.tile([128, 2, D + 1], f32, tag="o")
    # two transpose staging slots inside one PSUM bank (slices rotate)
    tps = psum_t.tile([128, 2, 128], bf16, tag="tp")

    # ---- PE warmup: dummy matmuls so the HAM clock-gate reaches 2.4 GHz
    # by the time real PE work arrives (identity transposes don't count).
    warm_src = singles.tile([128, 512], bf16, tag="warm_src")
    nc.vector.memset(warm_src[:], 0.0)

    def warm(n):
        # dummy matmuls into the PV bank (its first real use is far later;
        # writing a score buffer here would corrupt packed scores)
        for _ in range(n):
            nc.tensor.matmul(
                ops_tri[:, 0, :], lhsT=warm_src[:, 0:128],
                rhs=warm_src[:, 0:D + 1], start=True, stop=True,
            )

    warm(14)

    ident = singles.tile([128, 128], bf16)
    masks.make_identity(nc, ident[:])
    keep = singles.tile([128, 128], bf16)
    masks.make_upper_triangular(nc, keep[:], val=1.0, diag=True)

    qT = [None] * QH
    # heads 1-3 Q^T is produced by DMA engines in the background: a SWDGE
    # DMA casts fp32->bf16 (DRAM->DRAM), then an XBAR-transpose DMA lands
    # [d, seq] bf16 in SBUF. Issued staggered from inside the head loop so
    # they never compete with the critical k/q0/v loads.
    q_sc = [None] * QH
    for h in range(1, QH):
        q_sc[h] = nc.dram_tensor(f"q_sc{h}", [SEQ, D], bf16)

    def qcast(h):
        nc.gpsimd.dma_start(out=q_sc[h][:, :], in_=q[:, h * D:(h + 1) * D])

    # ---- fast-path prep: K and head-0 Q via load + DVE cast + PE transpose
    kT = singles.tile([128, SEQ], bf16, tag="kT")
    knat = singles.tile([128, NT, 128], f32, tag="knat")
    knat_bf = singles.tile([128, NT, 128], bf16, tag="knat_bf")
    kr = k.rearrange("(t p) d -> p t d", p=128)
    qT[0] = singles.tile([128, SEQ], bf16, tag="qT0", name="qT0")
    q0nat = singles.tile([128, NT, 128], f32, tag="q0nat")
    q0nat_bf = singles.tile([128, NT, 128], bf16, tag="q0nat_bf")
    q0r = q[:, 0:D].rearrange("(t p) d -> p t d", p=128)

    def prep_chunk(c, nat, nat_bf, dst, src, eng):
        cs = slice(c * 4, (c + 1) * 4)
        # loads split across both HWDGE engines (sync + scalar) — two
        # independent DMA rings, ~2x the effective load bandwidth
        eng.dma_start(out=nat[:, cs, :], in_=src[:, cs, :])
        nc.vector.tensor_copy(nat_bf[:, cs, :], nat[:, cs, :])
        for t in range(c * 4, (c + 1) * 4):
            pst = tps[:, t % 2, :]
            nc.tensor.transpose(pst, nat_bf[:, t, :], ident[:])
            nc.vector.tensor_copy(dst[:, t * 128:(t + 1) * 128], pst)

    # ---- V: natural [128, t, d] bf16 + ones column for the denominator
    vp = singles.tile([128, NT, D + 1], bf16)
    vr = v.rearrange("(t p) d -> p t d", p=128)

    # V: SWDGE cast DMA straight into SBUF bf16, issued at t=0 on the
    # otherwise-idle gpsimd ring; done before PV(qt=0) needs it.
    nc.gpsimd.dma_start(out=vp[:, :, 0:D], in_=vr)
    nc.vector.memset(vp[:, :, D:D + 1], 1.0)

    # Lazy prep: k/q0 chunks are emitted just-in-time from inside head
    # 0's buffer walk, so the first QK matmuls are not queued behind the
    # whole prep in the in-order PE stream.
    prep_state = {"k": 0, "q0": 0}

    def need_k(kt):
        while prep_state["k"] * 4 <= kt:
            prep_chunk(prep_state["k"], knat, knat_bf, kT, kr, nc.sync)
            warm(1)
            prep_state["k"] += 1

    def need_q0(qhi):
        while prep_state["q0"] * 512 < qhi:
            prep_chunk(prep_state["q0"], q0nat, q0nat_bf, qT[0], q0r, nc.scalar)
            warm(1)
            prep_state["q0"] += 1

    for h in range(1, QH):
        qT[h] = singles.tile([128, SEQ], bf16, tag=f"qT{h}", name=f"qT{h}")

    def qtrans(h):
        nc.sync.dma_start(out=qT[h][:, :], in_=q_sc[h][:, :], transpose=True)

    # (h, buffer) points at which to launch the background q prep
    PREP_EVENTS = {
        (0, 0): lambda: qcast(1),
        (0, 5): lambda: qcast(2),
        (0, 8): lambda: qtrans(1),
        (1, 1): lambda: qcast(3),
        (1, 5): lambda: qtrans(2),
        (2, 3): lambda: qtrans(3),
    }

    def emit_pv(h, qt, pT, osb):
        """O[qt] = sum_k2 (P^T slice).T @ [V | 1], then normalize + store."""
        ops = ops_tri[:, qt % 2, :]
        for k2 in range(qt + 1):
            c0 = ROFF[k2] + (qt - k2) * 128
            nc.tensor.matmul(
                ops,
                lhsT=pT[:, c0:c0 + 128],
                rhs=vp[:, k2, :],
                start=(k2 == 0),
                stop=(k2 == qt),
            )
        rec = opool.tile([128, 1], f32, tag="rec")
        nc.vector.reciprocal(rec[:], ops[:, D:D + 1])
        nc.vector.tensor_scalar_mul(osb[:, qt % 2, :], ops[:, 0:D], rec[:])
        if qt % 2 == 1:
            qb = qt // 2
            nc.sync.dma_start(
                out=out[qb * 256:(qb + 1) * 256, h * D:(h + 1) * D].rearrange(
                    "(j p) d -> p j d", p=128
                ),
                in_=osb[:],
            )

    # Pending-PV queue, flowing across head boundaries.
    pvq = []
    pv_state = {}

    def pop_pv():
        h2, qt2, pT2 = pvq.pop(0)
        st = pv_state.setdefault(h2, {})
        if qt2 % 2 == 0:
            st["osb"] = opool.tile([128, 2, D], f32, tag="osb", name="osb")
        emit_pv(h2, qt2, pT2, st["osb"])

    LAG = 4

    def region_of(c):
        kt = 0
        while ROFF[kt + 1] <= c:
            kt += 1
        return kt

    for h in range(QH):
        pT = ppool.tile([128, PCOLS], bf16, tag="pT")
        next_qt = 0  # next query tile to mark PV-ready
        for bi, (b0, bsz, which) in enumerate(BUFS):
            ev = PREP_EVENTS.get((h, bi))
            if ev is not None:
                ev()
            # drain PV backlog first: gives the scalar engine time to free
            # the psum buffer this iteration is about to overwrite
            while len(pvq) > LAG:
                pop_pv()
            sbuf_tile = sA if which == 0 else sB
            # exact-causal QK chunks packed into this psum buffer
            c = b0
            while c < b0 + bsz:
                kt = region_of(c)
                qoff = kt * 128 + (c - ROFF[kt])  # query index of col c
                step = min(
                    512 - (c - b0) % 512,  # psum bank grid
                    ROFF[kt + 1] - c,      # region end
                    b0 + bsz - c,          # buffer end
                )
                if h == 0:
                    need_k(kt)
                    need_q0(qoff + step)
                nc.tensor.matmul(
                    sbuf_tile[:, c - b0:c - b0 + step],
                    lhsT=kT[:, kt * 128:(kt + 1) * 128],
                    rhs=qT[h][:, qoff:qoff + step],
                    start=True,
                    stop=True,
                )
                c += step
            # one wide exp for the whole buffer
            nc.scalar.activation(
                pT[:, b0:b0 + bsz], sbuf_tile[:, 0:bsz], Exp, scale=SCALE
            )
            # mask any diagonal block this buffer completed (on GpSimd)
            kt = region_of(b0)
            while kt < NT and ROFF[kt] + 128 <= b0 + bsz:
                if ROFF[kt] + 128 > b0:
                    nc.gpsimd.tensor_mul(
                        pT[:, ROFF[kt]:ROFF[kt] + 128],
                        pT[:, ROFF[kt]:ROFF[kt] + 128],
                        keep[:],
                    )
                kt += 1
            # queue query tiles whose last dependency (diag block) is done
            while next_qt < NT and ROFF[next_qt] + 128 <= b0 + bsz:
                pvq.append((h, next_qt, pT))
                next_qt += 1
    while pvq:
        pop_pv()


def _build():
    import concourse.mybir as mybir
    import concourse.tile as tile
    from concourse import bacc
    from contextlib import ExitStack

    nc = bacc.Bacc()
    q = nc.declare_dram_parameter("q", [SEQ, QH * D], mybir.dt.float32, isOutput=False)
    k = nc.declare_dram_parameter("k", [SEQ, D], mybir.dt.float32, isOutput=False)
    v = nc.declare_dram_parameter("v", [SEQ, D], mybir.dt.float32, isOutput=False)
    out = nc.declare_dram_parameter("out", [SEQ, QH * D], mybir.dt.float32, isOutput=True)

    with tile.TileContext(nc) as tc:
        with ExitStack() as ctx:
            _emit(ctx, tc, q[:], k[:], v[:], out[:])
    nc.compile()
    return nc


def _get_nc():
    global _NC
    if _NC is None:
        _NC = _build()
    return _NC


def _ensure_ntff_hook():
    """The agent image's antenv lacks axon_hooks; shim it so trace=True works."""
    import sys
    import types

    if "antenv.axon_hooks" in sys.modules:
        return
    try:
        import antenv
        from trn_agent_boot.trn_boot import _ntff_profile_via_ctypes
    except ImportError:
        return
    mod = types.ModuleType("antenv.axon_hooks")
    hook = [None]
    mod.set_axon_ntff_profile_hook = lambda h: hook.__setitem__(0, h)
    mod.get_axon_ntff_profile_hook = lambda: hook[0]
    sys.modules["antenv.axon_hooks"] = mod
    antenv.axon_hooks = mod
    mod.set_axon_ntff_profile_hook(_ntff_profile_via_ctypes("/opt/axon/libaxon_pjrt.so"))


def _run(q, k, v, trace=False):
    from concourse.bass_utils import run_bass_kernel_spmd

    if trace:
        _ensure_ntff_hook()
    nc = _get_nc()
    in_maps = []
    for i in range(N_CORES):
        in_maps.append(
            {
                "q": np.ascontiguousarray(q[:, i * QH * D:(i + 1) * QH * D]).astype(np.float32, copy=False),
                "k": np.ascontiguousarray(k[:, i * D:(i + 1) * D]).astype(np.float32, copy=False),
                "v": np.ascontiguousarray(v[:, i * D:(i + 1) * D]).astype(np.float32, copy=False),
            }
        )
    res = run_bass_kernel_spmd(nc, in_maps, core_ids=list(range(N_CORES)), trace=trace)
    full = np.concatenate([res.results[i]["out"] for i in range(N_CORES)], axis=1)
    return full.astype(np.float32, copy=False), res


def kernel(q, k, v):
    out, _ = _run(q, k, v, trace=False)
    return out


# revision 25
# speedup vs baseline: 1.0659x; 1.0659x over previous
"""GQA causal attention (S=2048, H=32, KVH=8, D=128) on 8 TRN2 NeuronCores.

Sharding: tensor-parallel over heads. Core i computes query heads
[4i, 4i+4) against KV head i (GQA group size 32/8 = 4). No collectives:
the host slices the inputs per core and concatenates the outputs.

Per-core algorithm (seq=2048, d=128, 4 q-heads, 1 kv-head, causal):
  - K^T and head-0 Q^T take the low-latency prep path: fp32 DMA load,
    DVE cast to bf16, PE identity-matmul transpose (chunked, interleaved
    with warmup matmuls that hold the HAM clock at 2.4 GHz).
  - Q^T for heads 1-3 is produced entirely by DMA engines in the
    background: a SWDGE DMA casts fp32->bf16 (DRAM->DRAM), then an
    XBAR-transpose DMA lands [d=128, seq] bf16 in SBUF. These are issued
    at t=0 and consumed 25+ us later.
  - Per head, exact-causal score tiles S^T[kt] = K_tile^T @ Q^T (only
    q >= kt*128) are written PACKED into alternating PSUM buffers
    A[128,2048] / B[128,1024]; ONE wide ACTIVATE(Exp, scale) per buffer
    writes into the packed P^T buffer [128, 17408] bf16 (scores are O(1)
    so no max subtraction). 44 activations instead of 96.
  - The diagonal 128-col block of each key-tile region is masked by a
    0/1 upper-triangular multiply on the (otherwise idle) GpSimd engine.
  - PV: for each query tile qt, acc[qt] = sum_k2 (P^T slice).T @ [V | 1]
    accumulated in PSUM; column 128 is the softmax denominator.
    DVE reciprocal + tensor_scalar_mul normalizes; one DMA per 256 rows
    stores the result. PV lags the QK/exp pipeline by a few query tiles
    and flows across head boundaries so no engine sees a bubble.
"""

import numpy as np

SEQ = 2048
D = 128
QH = 4  # query heads per core
N_CORES = 8
SCALE = 0.08838834764831845  # 1/sqrt(128)
NT = SEQ // 128  # 16 tiles of 128 along seq

_NC = None

# packed score-column layout (identical per head)
ROFF = [0]
for _kt in range(1, NT + 1):
    ROFF.append(ROFF[-1] + (SEQ - 128 * (_kt - 1)))
PCOLS = ROFF[NT]  # 17408

# psum buffers: B(1024) first so the head's first ACTIVATE has a short
# dependency chain, then alternate with A(2048); the tail is one extra A.
_SIZES = [1024, 2048] * 5 + [2048]  # sums to PCOLS
BUFS = []
_c = 0
for _sz in _SIZES:
    BUFS.append((_c, _sz, 1 if _sz == 1024 else 0))  # (start, size, pool: 0=A,1=B)
    _c += _sz
assert _c == PCOLS


def _emit(ctx, tc, q, k, v, out):
    import concourse.mybir as mybir
    from concourse import masks

    nc = tc.nc
    f32 = mybir.dt.float32
    bf16 = mybir.dt.bfloat16
    Exp = mybir.ActivationFunctionType.Exp

    singles = ctx.enter_context(tc.tile_pool(name="singles", bufs=1))
    ppool = ctx.enter_context(tc.tile_pool(name="ppool", bufs=2))
    opool = ctx.enter_context(tc.tile_pool(name="opool", bufs=3))
    # PSUM budget (8 banks = 16KB/partition):
    #   A 2048 f32 = 4 banks, B 1024 f32 = 2 banks,
    #   PV acc [128,2,129] f32 = 1 bank, transpose staging = 1 bank
    psum_a = ctx.enter_context(tc.tile_pool(name="psum_a", bufs=1, space="PSUM"))
    psum_b = ctx.enter_context(tc.tile_pool(name="psum_b", bufs=1, space="PSUM"))
    psum_o = ctx.enter_context(tc.tile_pool(name="psum_o", bufs=1, space="PSUM"))
    psum_t = ctx.enter_context(tc.tile_pool(name="psum_t", bufs=1, space="PSUM"))

    sA = psum_a.tile([128, 2048], f32, tag="A")
    sB = psum_b.tile([128, 1024], f32, tag="B")
    ops_tri = psum_o.tile([128, 2, D + 1], f32, tag="o")
    # two transpose staging slots inside one PSUM bank (slices rotate)
    tps = psum_t.tile([128, 2, 128], bf16, tag="tp")

    # ---- PE warmup: dummy matmuls so the HAM clock-gate reaches 2.4 GHz
    # by the time real PE work arrives (identity transposes don't count).
    warm_src = singles.tile([128, 512], bf16, tag="warm_src")
    nc.vector.memset(warm_src[:], 0.0)

    def warm(n):
        # dummy matmuls into the PV bank (its first real use is far later;
        # writing a score buffer here would corrupt packed scores)
        for _ in range(n):
            nc.tensor.matmul(
                ops_tri[:, 0, :], lhsT=warm_src[:, 0:128],
                rhs=warm_src[:, 0:D + 1], start=True, stop=True,
            )

    # sustained back-to-back burst: HAM needs ~3.4us of continuous PE
    # activity to lift the clock gate to 2.4 GHz (transposes don't count)
    warm(28)

    ident = singles.tile([128, 128], bf16)
    masks.make_identity(nc, ident[:])
    keep = singles.tile([128, 128], bf16)
    masks.make_upper_triangular(nc, keep[:], val=1.0, diag=True)

    qT = [None] * QH
    # heads 1-3 Q^T is produced by DMA engines in the background: a SWDGE
    # DMA casts fp32->bf16 (DRAM->DRAM), then an XBAR-transpose DMA lands
    # [d, seq] bf16 in SBUF. Issued staggered from inside the head loop so
    # they never compete with the critical k/q0/v loads.
    q_sc = [None] * QH
    for h in range(1, QH):
        q_sc[h] = nc.dram_tensor(f"q_sc{h}", [SEQ, D], bf16)

    def qcast(h):
        nc.gpsimd.dma_start(out=q_sc[h][:, :], in_=q[:, h * D:(h + 1) * D])

    # ---- fast-path prep: K and head-0 Q via load + DVE cast + PE transpose
    kT = singles.tile([128, SEQ], bf16, tag="kT")
    knat = singles.tile([128, NT, 128], f32, tag="knat")
    knat_bf = singles.tile([128, NT, 128], bf16, tag="knat_bf")
    kr = k.rearrange("(t p) d -> p t d", p=128)
    qT[0] = singles.tile([128, SEQ], bf16, tag="qT0", name="qT0")
    q0nat = singles.tile([128, NT, 128], f32, tag="q0nat")
    q0nat_bf = singles.tile([128, NT, 128], bf16, tag="q0nat_bf")
    q0r = q[:, 0:D].rearrange("(t p) d -> p t d", p=128)

    def prep_chunk(c, nat, nat_bf, dst, src, eng):
        cs = slice(c * 4, (c + 1) * 4)
        # loads split across both HWDGE engines (sync + scalar) — two
        # independent DMA rings, ~2x the effective load bandwidth
        eng.dma_start(out=nat[:, cs, :], in_=src[:, cs, :])
        nc.vector.tensor_copy(nat_bf[:, cs, :], nat[:, cs, :])
        for t in range(c * 4, (c + 1) * 4):
            pst = tps[:, t % 2, :]
            nc.tensor.transpose(pst, nat_bf[:, t, :], ident[:])
            nc.vector.tensor_copy(dst[:, t * 128:(t + 1) * 128], pst)
            if t % 2 == 1:
                warm(1)  # transposes alone don't keep HAM at 2.4 GHz

    # ---- V: natural [128, t, d] bf16 + ones column for the denominator
    vp = singles.tile([128, NT, D + 1], bf16)
    vr = v.rearrange("(t p) d -> p t d", p=128)

    vnat = singles.tile([128, NT, 128], f32, tag="vnat")
    nc.vector.memset(vp[:, :, D:D + 1], 1.0)

    def vchunk(c):
        cs = slice(c * 8, (c + 1) * 8)
        nc.sync.dma_start(out=vnat[:, cs, :], in_=vr[:, cs, :])
        nc.vector.tensor_copy(vp[:, cs, 0:D], vnat[:, cs, :])

    # Lazy prep: k/q0/v chunks are emitted just-in-time from inside head
    # 0's buffer walk, so the first QK matmuls are not queued behind the
    # whole prep in the in-order PE stream. V rides the sync ring between
    # K chunks (it is needed by PV from ~qt=0 onward).
    prep_state = {"k": 0, "q0": 0}

    def need_k(kt):
        while prep_state["k"] * 4 <= kt:
            c = prep_state["k"]
            prep_chunk(c, knat, knat_bf, kT, kr, nc.sync)
            warm(1)
            if c < 2:
                vchunk(c)
            prep_state["k"] += 1

    def need_q0(qhi):
        while prep_state["q0"] * 512 < qhi:
            prep_chunk(prep_state["q0"], q0nat, q0nat_bf, qT[0], q0r, nc.scalar)
            warm(1)
            prep_state["q0"] += 1

    for h in range(1, QH):
        qT[h] = singles.tile([128, SEQ], bf16, tag=f"qT{h}", name=f"qT{h}")

    def qtrans(h):
        nc.sync.dma_start(out=qT[h][:, :], in_=q_sc[h][:, :], transpose=True)

    # (h, buffer) points at which to launch the background q prep
    PREP_EVENTS = {
        (0, 0): lambda: qcast(1),
        (0, 5): lambda: qcast(2),
        (0, 8): lambda: qtrans(1),
        (1, 1): lambda: qcast(3),
        (1, 5): lambda: qtrans(2),
        (2, 3): lambda: qtrans(3),
    }

    def emit_pv(h, qt, pT, osb):
        """O[qt] = sum_k2 (P^T slice).T @ [V | 1], then normalize + store."""
        ops = ops_tri[:, qt % 2, :]
        for k2 in range(qt + 1):
            c0 = ROFF[k2] + (qt - k2) * 128
            nc.tensor.matmul(
                ops,
                lhsT=pT[:, c0:c0 + 128],
                rhs=vp[:, k2, :],
                start=(k2 == 0),
                stop=(k2 == qt),
            )
        rec = opool.tile([128, 1], f32, tag="rec")
        nc.vector.reciprocal(rec[:], ops[:, D:D + 1])
        nc.vector.tensor_scalar_mul(osb[:, qt % 2, :], ops[:, 0:D], rec[:])
        if qt % 2 == 1:
            qb = qt // 2
            nc.sync.dma_start(
                out=out[qb * 256:(qb + 1) * 256, h * D:(h + 1) * D].rearrange(
                    "(j p) d -> p j d", p=128
                ),
                in_=osb[:],
            )

    # Pending-PV queue, flowing across head boundaries.
    pvq = []
    pv_state = {}

    def pop_pv():
        h2, qt2, pT2 = pvq.pop(0)
        st = pv_state.setdefault(h2, {})
        if qt2 % 2 == 0:
            st["osb"] = opool.tile([128, 2, D], f32, tag="osb", name="osb")
        emit_pv(h2, qt2, pT2, st["osb"])

    LAG = 4

    def region_of(c):
        kt = 0
        while ROFF[kt + 1] <= c:
            kt += 1
        return kt

    for h in range(QH):
        pT = ppool.tile([128, PCOLS], bf16, tag="pT")
        next_qt = 0  # next query tile to mark PV-ready
        for bi, (b0, bsz, which) in enumerate(BUFS):
            ev = PREP_EVENTS.get((h, bi))
            if ev is not None:
                ev()
            # drain PV backlog first: gives the scalar engine time to free
            # the psum buffer this iteration is about to overwrite
            while len(pvq) > LAG:
                pop_pv()
            sbuf_tile = sA if which == 0 else sB
            # exact-causal QK chunks packed into this psum buffer
            c = b0
            while c < b0 + bsz:
                kt = region_of(c)
                qoff = kt * 128 + (c - ROFF[kt])  # query index of col c
                step = min(
                    512 - (c - b0) % 512,  # psum bank grid
                    ROFF[kt + 1] - c,      # region end
                    b0 + bsz - c,          # buffer end
                )
                if h == 0:
                    need_k(kt)
                    need_q0(qoff + step)
                nc.tensor.matmul(
                    sbuf_tile[:, c - b0:c - b0 + step],
                    lhsT=kT[:, kt * 128:(kt + 1) * 128],
                    rhs=qT[h][:, qoff:qoff + step],
                    start=True,
                    stop=True,
                )
                c += step
            # one wide exp for the whole buffer
            nc.scalar.activation(
                pT[:, b0:b0 + bsz], sbuf_tile[:, 0:bsz], Exp, scale=SCALE
            )
            # mask any diagonal block this buffer completed (on GpSimd)
            kt = region_of(b0)
            while kt < NT and ROFF[kt] + 128 <= b0 + bsz:
                if ROFF[kt] + 128 > b0:
                    nc.gpsimd.tensor_mul(
                        pT[:, ROFF[kt]:ROFF[kt] + 128],
                        pT[:, ROFF[kt]:ROFF[kt] + 128],
                        keep[:],
                    )
                kt += 1
            # queue query tiles whose last dependency (diag block) is done
            while next_qt < NT and ROFF[next_qt] + 128 <= b0 + bsz:
                pvq.append((h, next_qt, pT))
                next_qt += 1
    while pvq:
        pop_pv()


def _build():
    import concourse.mybir as mybir
    import concourse.tile as tile
    from concourse import bacc
    from contextlib import ExitStack

    nc = bacc.Bacc()
    q = nc.declare_dram_parameter("q", [SEQ, QH * D], mybir.dt.float32, isOutput=False)
    k = nc.declare_dram_parameter("k", [SEQ, D], mybir.dt.float32, isOutput=False)
    v = nc.declare_dram_parameter("v", [SEQ, D], mybir.dt.float32, isOutput=False)
    out = nc.declare_dram_parameter("out", [SEQ, QH * D], mybir.dt.float32, isOutput=True)

    with tile.TileContext(nc) as tc:
        with ExitStack() as ctx:
            _emit(ctx, tc, q[:], k[:], v[:], out[:])
    nc.compile()
    return nc


def _get_nc():
    global _NC
    if _NC is None:
        _NC = _build()
    return _NC


def _ensure_ntff_hook():
    """The agent image's antenv lacks axon_hooks; shim it so trace=True works."""
    import sys
    import types

    if "antenv.axon_hooks" in sys.modules:
        return
    try:
        import antenv
        from trn_agent_boot.trn_boot import _ntff_profile_via_ctypes
    except ImportError:
        return
    mod = types.ModuleType("antenv.axon_hooks")
    hook = [None]
    mod.set_axon_ntff_profile_hook = lambda h: hook.__setitem__(0, h)
    mod.get_axon_ntff_profile_hook = lambda: hook[0]
    sys.modules["antenv.axon_hooks"] = mod
    antenv.axon_hooks = mod
    mod.set_axon_ntff_profile_hook(_ntff_profile_via_ctypes("/opt/axon/libaxon_pjrt.so"))


def _run(q, k, v, trace=False):
    from concourse.bass_utils import run_bass_kernel_spmd

    if trace:
        _ensure_ntff_hook()
    nc = _get_nc()
    in_maps = []
    for i in range(N_CORES):
        in_maps.append(
            {
                "q": np.ascontiguousarray(q[:, i * QH * D:(i + 1) * QH * D]).astype(np.float32, copy=False),
                "k": np.ascontiguousarray(k[:, i * D:(i + 1) * D]).astype(np.float32, copy=False),
                "v": np.ascontiguousarray(v[:, i * D:(i + 1) * D]).astype(np.float32, copy=False),
            }
        )
    res = run_bass_kernel_spmd(nc, in_maps, core_ids=list(range(N_CORES)), trace=trace)
    full = np.concatenate([res.results[i]["out"] for i in range(N_CORES)], axis=1)
    return full.astype(np.float32, copy=False), res


def kernel(q, k, v):
    out, _ = _run(q, k, v, trace=False)
    return out
